# revision 2
# baseline (speedup 1.0000x reference)
"""Luong attention kernel v3 for Trainium2 (Bass/Tile), data-parallel over batch.

Math (per batch b, one core each):
    scores[s,t] = enc[s,:] . dec[t,:]
    weights     = softmax(scores, axis=t)
    context[s]  = sum_t weights[s,t] * enc[t,:]
    out         = tanh(concat([context, dec]) @ W_tanh)

v3 key decisions (driven by HW probes on this platform):
  - NO xbar DMA transposes (they serialize against the whole pipeline here)
    and NO DRAM f16 bounce: enc/dec are cast-loaded f32->f16 straight into
    SBUF by SWDGE (gpsimd) with a contiguous-per-partition "(p n) d" layout,
    and the [d, t]/[d, s] operands are produced by PE transpose-mode matmuls
    (batched 8 per PSUM bank in one accumulation group) + DVE copies.
  - Both t and s are indexed interleaved: index (p, n) <-> row p*16+n. All
    matmul chunk slices (scores lhsT/rhs, U lhsT/rhs, phase-3 lhsT, fold
    lhsT) and the output DMA are then fully contiguous.
  - exp batched N=1024 from 2-bank PSUM scores tiles; U matmuls interleaved
    between score quads (lag 2) so the PE fills exp-wait gaps.
  - Esum via a 4-level slab tree in bf16 on DVE; denominator folded across
    partitions by a single 16-matmul PE group into a shared PSUM bank ring;
    softmax normalization deferred to phase 3 (per-partition 1/denom).
  - Total per-rep DMA: 4MB in + 2MB out, all contiguous per partition.
"""

import sys

if "/opt/trn_rl_repo" not in sys.path:
    sys.path.insert(0, "/opt/trn_rl_repo")

import numpy as np

import concourse.bacc as bacc
import concourse.mybir as mybir
import concourse.tile as tile
import concourse.masks as masks
from concourse import bass_utils

B, S, D = 8, 2048, 256
P = 128
NT = S // P  # 16 groups; row index (p, n) <-> p*16 + n
SB = 512  # s-block = 4 n-slots x 128
NSB = S // SB  # 4
DC = D // P  # 2 feature chunks
TQ = 2  # t-groups per exp batch (N=1024, 2 PSUM banks)
SHIFT = 64.0

_CACHE = {}


def _build(reps: int = 1, phases: str = "fsup"):
    f32, bf16, f16 = mybir.dt.float32, mybir.dt.bfloat16, mybir.dt.float16
    AF = mybir.ActivationFunctionType

    nc = bacc.Bacc("TRN2", target_bir_lowering=False, debug=False)
    enc_d = nc.dram_tensor("enc", [S, D], f32, kind="ExternalInput").ap()
    dec_d = nc.dram_tensor("dec", [S, D], f32, kind="ExternalInput").ap()
    w_d = nc.dram_tensor("w", [2 * D, D], f32, kind="ExternalInput").ap()
    out_d = nc.dram_tensor("out", [S, D], f32, kind="ExternalOutput").ap()

    with tile.TileContext(nc) as tc:
        with (
            tc.tile_pool(name="pers", bufs=1) as pers,
            tc.tile_pool(name="front", bufs=2) as front,
            tc.tile_pool(name="work", bufs=2) as work,
            tc.tile_pool(name="outp", bufs=2) as outp,
            tc.tile_pool(name="ps_s", bufs=2, space="PSUM") as ps_s,
            tc.tile_pool(name="ps_u", bufs=2, space="PSUM") as ps_u,
            tc.tile_pool(name="ps_t", bufs=2, space="PSUM") as ps_t,
        ):
            ones = pers.tile([P, 1], bf16, tag="ones")
            nshift = pers.tile([P, 1], f32, tag="nshift")
            zbias = pers.tile([P, 1], f32, tag="zbias")
            ident = pers.tile([P, P], f16, tag="ident")
            EsumB = pers.tile([P, NT, P], bf16, tag="EsumB")
            U = pers.tile([P, DC, NT, P], bf16, tag="U")
            Wt1 = pers.tile([P, DC, D], bf16, tag="Wt1")
            Wt2 = pers.tile([P, DC, D], f16, tag="Wt2")

            nc.any.memset(ones[:], 1.0)
            nc.any.memset(nshift[:], -SHIFT)
            nc.any.memset(zbias[:], 0.0)
            masks.make_identity(nc, ident[:])

            # ---- W (constant): one load + DVE casts, outside the rep loop
            wst = pers.tile([P, 4, D], f32, tag="wst")
            nc.sync.dma_start(wst[:], w_d.rearrange("(r p) d -> p r d", p=P))
            for r in range(2):
                nc.vector.tensor_copy(Wt1[:, r, :], wst[:, r, :])
                nc.vector.tensor_copy(Wt2[:, r, :], wst[:, 2 + r, :])

            def emit_casts():
                # f32 -> f16 cast-loads straight to SBUF; partition p holds
                # rows p*16..p*16+15 (fully contiguous both sides).
                decN = front.tile([P, NT, D], f16, tag="decN")
                encN = front.tile([P, NT, D], f16, tag="encN")
                nc.gpsimd.dma_start(decN[:], dec_d.rearrange("(p n) d -> p n d", p=P))
                nc.gpsimd.dma_start(encN[:], enc_d.rearrange("(p n) d -> p n d", p=P))
                return decN, encN

            def emit_transposes(ft):
                # PE transpose-mode: xN[:, n, dc*128:(dc+1)*128] -> [d, p]
                # chunks, 8 per PSUM bank in ONE accumulation group, then one
                # DVE copy per bank.  xTi[:, dc, n, j] = x[j*16+n, dc*128+p].
                decN, encN = ft
                decTi = front.tile([P, DC, NT, P], f16, tag="decTi")
                encTi = front.tile([P, DC, NT, P], f16, tag="encTi")
                for src, dst in ((decN, decTi), (encN, encTi)):
                    for dc in range(DC):
                        for h in range(2):
                            pt = ps_t.tile([P, 8, P], f16, tag="pt")
                            for j in range(8):
                                n = h * 8 + j
                                nc.tensor.matmul(
                                    pt[:, j, :],
                                    src[:, n, dc * P : (dc + 1) * P],
                                    ident[:],
                                    is_transpose=True,
                                    start=(j == 0),
                                    stop=(j == 7),
                                )
                            nc.vector.tensor_copy(
                                dst[:, dc, h * 8 : (h + 1) * 8, :], pt[:]
                            )
                return decTi, encTi

            ft_cur = emit_casts()
            tp_cur = emit_transposes(ft_cur) if "s" in phases else None
            for _rep in range(reps):
                decN, encN = ft_cur
                if tp_cur is not None:
                    decTi, encTi = tp_cur
                if _rep + 1 < reps:
                    ft_next = emit_casts()

                rden = front.tile([P, NT], f32, tag="rden")
                for sb in range(NSB) if "s" in phases else []:
                    Eb = work.tile([P, NT, SB], bf16, tag="Eb")
                    pu0 = ps_u.tile([P, SB], f32, tag="pu")
                    pu1 = ps_u.tile([P, SB], f32, tag="pu")
                    pu = (pu0, pu1)
                    NQ = NT // TQ

                    def emit_u_quad(q):
                        for tin in range(TQ):
                            n = q * TQ + tin
                            for dc in range(DC):
                                nc.tensor.matmul(
                                    pu[dc][:],
                                    encN[:, n, dc * P : (dc + 1) * P],
                                    Eb[:, n, :],
                                    start=(n == 0),
                                    stop=(n == NT - 1),
                                )

                    # scores+exp with U matmuls interleaved (lag 1 group):
                    # the single 4-bank ps tile ping-pongs via exp completion;
                    # the PE fills the exp wait with the previous group's U.
                    for q in range(NQ):
                        ps = ps_s.tile([P, TQ, SB], f32, tag="ps")
                        for tin in range(TQ):
                            n = q * TQ + tin
                            for dc in range(DC):
                                nc.tensor.matmul(
                                    ps[:, tin, :],
                                    decTi[:, dc, n, :],
                                    encTi[:, dc, 4 * sb : 4 * sb + 4, :],
                                    start=(dc == 0),
                                    stop=(dc == DC - 1),
                                )
                        nc.scalar.activation(
                            Eb[:, q * TQ : (q + 1) * TQ, :],
                            ps[:],
                            AF.Exp,
                            bias=nshift[:],
                        )
                        if q >= 2 and "u" in phases:
                            emit_u_quad(q - 2)
                    if "u" not in phases:
                        continue
                    emit_u_quad(NQ - 2)
                    emit_u_quad(NQ - 1)
                    for dc in range(DC):
                        nc.vector.tensor_copy(
                            U[:, dc, 4 * sb : 4 * sb + 4, :], pu[dc][:]
                        )
                    # Esum slab tree (bf16)
                    T8 = work.tile([P, 8, SB], bf16, tag="T8")
                    T4 = work.tile([P, 4, SB], bf16, tag="T4")
                    T2 = work.tile([P, 2, SB], bf16, tag="T2")
                    nc.vector.tensor_add(T8[:], Eb[:, 0:8, :], Eb[:, 8:16, :])
                    nc.vector.tensor_add(T4[:], T8[:, 0:4, :], T8[:, 4:8, :])
                    nc.vector.tensor_add(T2[:], T4[:, 0:2, :], T4[:, 2:4, :])
                    nc.vector.tensor_add(
                        EsumB[:, 4 * sb : 4 * sb + 4, :], T2[:, 0, :], T2[:, 1, :]
                    )

                # next rep's transposes here: they fill the PE gap while the
                # DVE finishes the last Esum tree (fold depends on it).
                if _rep + 1 < reps:
                    tp_cur = emit_transposes(ft_next)
                    ft_cur = ft_next

                outS = outp.tile([P, NT, D], f32, tag="outS")
                if "p" not in phases:
                    nc.any.memset(outS[:], 0.1)
                    nc.sync.dma_start(
                        out_d.rearrange("(p n) d -> p n d", p=P), outS[:]
                    )
                    continue

                # denominator: one 16-matmul group into a pu-ring bank
                ytmp = ps_u.tile([P, SB], f32, tag="pu")
                for c in range(NT):
                    nc.tensor.matmul(
                        ytmp[:, c : c + 1],
                        EsumB[:, c, :],
                        ones[:],
                        start=(c == 0),
                        stop=(c == NT - 1),
                    )
                nc.vector.reciprocal(rden[:], ytmp[:, 0:NT])

                # phase 3: out rows s=p*16+c: tanh(U_c^T@W1 * rden + dec_c@W2)
                # c-chunks processed in pairs: one Tanh (N=512) per pair.
                def emit_y(c):
                    y = ps_u.tile([P, SB], f32, tag="pu")  # y1 | y2, one group
                    for dc in range(DC):
                        nc.tensor.matmul(
                            y[:, 0:D],
                            U[:, dc, c, :],
                            Wt1[:, dc, :],
                            start=(dc == 0),
                            stop=False,
                        )
                    for dc in range(DC):
                        nc.tensor.matmul(
                            y[:, D : 2 * D],
                            decTi[:, dc, c, :],
                            Wt2[:, dc, :],
                            start=False,
                            stop=(dc == DC - 1),
                        )
                    return y

                for k in range(NT // 2):
                    ya = emit_y(2 * k)
                    yb = emit_y(2 * k + 1)
                    t1 = work.tile([P, 2, D], f32, tag="t1")
                    nc.vector.tensor_scalar_mul(
                        t1[:, 0, :], ya[:, 0:D], rden[:, 2 * k : 2 * k + 1]
                    )
                    nc.vector.tensor_scalar_mul(
                        t1[:, 1, :], yb[:, 0:D], rden[:, 2 * k + 1 : 2 * k + 2]
                    )
                    t2 = work.tile([P, 2, D], f32, tag="t2")
                    nc.vector.tensor_add(t2[:, 0, :], t1[:, 0, :], ya[:, D : 2 * D])
                    nc.vector.tensor_add(t2[:, 1, :], t1[:, 1, :], yb[:, D : 2 * D])
                    nc.scalar.activation(
                        outS[:, 2 * k : 2 * k + 2, :], t2[:], AF.Tanh, bias=zbias[:]
                    )
                nc.sync.dma_start(
                    out_d.rearrange("(p n) d -> p n d", p=P), outS[:]
                )

    nc.compile()
    return nc


def get_nc():
    if "nc" not in _CACHE:
        _CACHE["nc"] = _build()
    return _CACHE["nc"]


def _get_fn():
    if "fn" in _CACHE:
        return _CACHE["fn"]
    import jax
    from jax.sharding import Mesh, NamedSharding, PartitionSpec
    from jax.experimental.shard_map import shard_map
    from concourse.bass2jax import (
        _bass_exec_p,
        install_neuronx_cc_hook,
        partition_id_tensor,
    )

    install_neuronx_cc_hook()
    nc = get_nc()
    out_avals = []
    for alloc in nc.m.functions[0].allocations:
        if (
            isinstance(alloc, mybir.MemoryLocationSet)
            and alloc.kind == "ExternalOutput"
        ):
            out_avals.append(
                jax.core.ShapedArray(
                    tuple(alloc.tensor_shape), mybir.dt.np(alloc.dtype)
                )
            )
    has_pid = nc.partition_id_tensor is not None
    names = ["enc", "dec", "w", "out"] + (["partition_id"] if has_pid else [])
    mesh = Mesh(np.asarray(jax.devices()[:B]), ("core",))
    spec = PartitionSpec("core")

    def _b(e, d, ww, z):
        ops = [e, d, ww, z] + ([partition_id_tensor()] if has_pid else [])
        return _bass_exec_p.bind(
            *ops,
            out_avals=tuple(out_avals),
            in_names=tuple(names),
            out_names=("out",),
            lowering_input_output_aliases=(),
            sim_require_finite=True,
            sim_require_nnan=True,
            nc=nc,
        )[0]

    jitted = jax.jit(
        shard_map(
            _b, mesh=mesh, in_specs=(spec,) * 4, out_specs=spec, check_rep=False
        ),
        donate_argnums=(3,),
        keep_unused=True,
    )
    sh = NamedSharding(mesh, spec)
    _CACHE["fn"] = (jitted, sh)
    return _CACHE["fn"]


def kernel(enc_outputs_top, dec_outputs_top, W_tanh):
    import jax

    enc = np.ascontiguousarray(enc_outputs_top, dtype=np.float32)
    dec = np.ascontiguousarray(dec_outputs_top, dtype=np.float32)
    w = np.ascontiguousarray(W_tanh, dtype=np.float32)
    try:
        fn, sh = _get_fn()
        eg = jax.device_put(enc.reshape(B * S, D), sh)
        dg = jax.device_put(dec.reshape(B * S, D), sh)
        wg = jax.device_put(np.concatenate([w] * B, axis=0), sh)
        zg = jax.device_put(np.zeros((B * S, D), np.float32), sh)
        out = np.asarray(jax.block_until_ready(fn(eg, dg, wg, zg)))
        return out.reshape(B, S, D)
    except Exception:
        nc = get_nc()
        in_maps = [{"enc": enc[b], "dec": dec[b], "w": w} for b in range(B)]
        res = bass_utils.run_bass_kernel_spmd(nc, in_maps, core_ids=list(range(B)))
        return np.stack([r["out"] for r in res.results], axis=0)


# revision 3
# speedup vs baseline: 1.4475x; 1.4475x over previous
"""Luong attention kernel v3 for Trainium2 (Bass/Tile), data-parallel over batch.

Math (per batch b, one core each):
    scores[s,t] = enc[s,:] . dec[t,:]
    weights     = softmax(scores, axis=t)
    context[s]  = sum_t weights[s,t] * enc[t,:]
    out         = tanh(concat([context, dec]) @ W_tanh)

v3 key decisions (driven by HW probes on this platform):
  - NO xbar DMA transposes (they serialize against the whole pipeline here)
    and NO DRAM f16 bounce: enc/dec are cast-loaded f32->f16 straight into
    SBUF by SWDGE (gpsimd) with a contiguous-per-partition "(p n) d" layout,
    and the [d, t]/[d, s] operands are produced by PE transpose-mode matmuls
    (batched 8 per PSUM bank in one accumulation group) + DVE copies.
  - Both t and s are indexed interleaved: index (p, n) <-> row p*16+n. All
    matmul chunk slices (scores lhsT/rhs, U lhsT/rhs, phase-3 lhsT, fold
    lhsT) and the output DMA are then fully contiguous.
  - exp batched N=1024 from 2-bank PSUM scores tiles; U matmuls interleaved
    between score quads (lag 2) so the PE fills exp-wait gaps.
  - Esum via a 4-level slab tree in bf16 on DVE; denominator folded across
    partitions by a single 16-matmul PE group into a shared PSUM bank ring;
    softmax normalization deferred to phase 3 (per-partition 1/denom).
  - Total per-rep DMA: 4MB in + 2MB out, all contiguous per partition.
"""

import sys

if "/opt/trn_rl_repo" not in sys.path:
    sys.path.insert(0, "/opt/trn_rl_repo")

import numpy as np

import concourse.bacc as bacc
import concourse.mybir as mybir
import concourse.tile as tile
import concourse.masks as masks
from concourse import bass_utils

B, S, D = 8, 2048, 256
P = 128
NT = S // P  # 16 groups; row index (p, n) <-> p*16 + n
SB = 512  # s-block = 4 n-slots x 128
NSB = S // SB  # 4
DC = D // P  # 2 feature chunks
TQ = 2  # t-groups per exp batch (N=1024, 2 PSUM banks)
SHIFT = 64.0

_CACHE = {}


def _build(reps: int = 1, phases: str = "fsup"):
    f32, bf16, f16 = mybir.dt.float32, mybir.dt.bfloat16, mybir.dt.float16
    AF = mybir.ActivationFunctionType

    nc = bacc.Bacc("TRN2", target_bir_lowering=False, debug=False)
    enc_d = nc.dram_tensor("enc", [S, D], f32, kind="ExternalInput").ap()
    dec_d = nc.dram_tensor("dec", [S, D], f32, kind="ExternalInput").ap()
    w_d = nc.dram_tensor("w", [2 * D, D], f32, kind="ExternalInput").ap()
    out_d = nc.dram_tensor("out", [S, D], f32, kind="ExternalOutput").ap()

    with tile.TileContext(nc) as tc:
        with (
            tc.tile_pool(name="pers", bufs=1) as pers,
            tc.tile_pool(name="front", bufs=2) as front,
            tc.tile_pool(name="work", bufs=2) as work,
            tc.tile_pool(name="outp", bufs=2) as outp,
            tc.tile_pool(name="ps_s", bufs=2, space="PSUM") as ps_s,
            tc.tile_pool(name="ps_u", bufs=2, space="PSUM") as ps_u,
            tc.tile_pool(name="ps_t", bufs=2, space="PSUM") as ps_t,
        ):
            ones = pers.tile([P, 1], bf16, tag="ones")
            nshift = pers.tile([P, 1], f32, tag="nshift")
            zbias = pers.tile([P, 1], f32, tag="zbias")
            ident = pers.tile([P, P], f16, tag="ident")
            EsumB = pers.tile([P, NT, P], bf16, tag="EsumB")
            U = pers.tile([P, DC, NT, P], bf16, tag="U")
            Wt1 = pers.tile([P, DC, D], bf16, tag="Wt1")
            Wt2 = pers.tile([P, DC, D], f16, tag="Wt2")

            nc.any.memset(ones[:], 1.0)
            nc.any.memset(nshift[:], -SHIFT)
            nc.any.memset(zbias[:], 0.0)
            masks.make_identity(nc, ident[:])

            # ---- W (constant): one load + DVE casts, outside the rep loop
            wst = pers.tile([P, 4, D], f32, tag="wst")
            nc.sync.dma_start(wst[:], w_d.rearrange("(r p) d -> p r d", p=P))
            for r in range(2):
                nc.vector.tensor_copy(Wt1[:, r, :], wst[:, r, :])
                nc.vector.tensor_copy(Wt2[:, r, :], wst[:, 2 + r, :])

            def emit_casts():
                # f32 -> f16 cast-loads straight to SBUF; partition p holds
                # rows p*16..p*16+15 (fully contiguous both sides).
                decN = front.tile([P, NT, D], f16, tag="decN")
                encN = front.tile([P, NT, D], f16, tag="encN")
                nc.gpsimd.dma_start(decN[:], dec_d.rearrange("(p n) d -> p n d", p=P))
                nc.gpsimd.dma_start(encN[:], enc_d.rearrange("(p n) d -> p n d", p=P))
                return decN, encN

            def make_transpose_emitters(ft):
                # PE transpose-mode: xN[:, n, dc*128:(dc+1)*128] -> [d, p]
                # chunks, 8 per PSUM bank in ONE accumulation group, then one
                # DVE copy per bank.  xTi[:, dc, n, j] = x[j*16+n, dc*128+p].
                # Returned as 8 per-group emitters so the groups can be spread
                # across the previous rep's sb iterations (avoids bunching the
                # copies in front of the phase-3 DVE chain).
                decNx, encNx = ft
                decTix = front.tile([P, DC, NT, P], f16, tag="decTi")
                encTix = front.tile([P, DC, NT, P], f16, tag="encTi")
                ems = []
                for srcx, dstx in ((decNx, decTix), (encNx, encTix)):
                    for dc in range(DC):
                        for h in range(2):
                            def em(srcx=srcx, dstx=dstx, dc=dc, h=h):
                                pt = ps_t.tile([P, 8, P], f16, tag="pt")
                                for j in range(8):
                                    n = h * 8 + j
                                    nc.tensor.matmul(
                                        pt[:, j, :],
                                        srcx[:, n, dc * P : (dc + 1) * P],
                                        ident[:],
                                        is_transpose=True,
                                        start=(j == 0),
                                        stop=(j == 7),
                                    )
                                nc.vector.tensor_copy(
                                    dstx[:, dc, h * 8 : (h + 1) * 8, :], pt[:]
                                )
                            ems.append(em)
                return (decTix, encTix), ems

            ft_cur = emit_casts()
            if "s" in phases:
                tp_cur, _ems0 = make_transpose_emitters(ft_cur)
                for _em in _ems0:
                    _em()
            else:
                tp_cur = None
            ems_next = []
            for _rep in range(reps):
                decN, encN = ft_cur
                if tp_cur is not None:
                    decTi, encTi = tp_cur
                if _rep + 1 < reps:
                    ft_next = emit_casts()
                    if "s" in phases:
                        tp_next, ems_next = make_transpose_emitters(ft_next)

                rden = front.tile([P, NT], f32, tag="rden")
                for sb in range(NSB) if "s" in phases else []:
                    Eb = work.tile([P, NT, SB], bf16, tag="Eb")
                    pu0 = ps_u.tile([P, SB], f32, tag="pu")
                    pu1 = ps_u.tile([P, SB], f32, tag="pu")
                    pu = (pu0, pu1)
                    NQ = NT // TQ

                    def emit_u_quad(q):
                        for tin in range(TQ):
                            n = q * TQ + tin
                            for dc in range(DC):
                                nc.tensor.matmul(
                                    pu[dc][:],
                                    encN[:, n, dc * P : (dc + 1) * P],
                                    Eb[:, n, :],
                                    start=(n == 0),
                                    stop=(n == NT - 1),
                                )

                    # scores+exp with U matmuls interleaved (lag 1 group):
                    # the single 4-bank ps tile ping-pongs via exp completion;
                    # the PE fills the exp wait with the previous group's U.
                    for q in range(NQ):
                        ps = ps_s.tile([P, TQ, SB], f32, tag="ps")
                        for tin in range(TQ):
                            n = q * TQ + tin
                            for dc in range(DC):
                                nc.tensor.matmul(
                                    ps[:, tin, :],
                                    decTi[:, dc, n, :],
                                    encTi[:, dc, 4 * sb : 4 * sb + 4, :],
                                    start=(dc == 0),
                                    stop=(dc == DC - 1),
                                )
                        nc.scalar.activation(
                            Eb[:, q * TQ : (q + 1) * TQ, :],
                            ps[:],
                            AF.Exp,
                            bias=nshift[:],
                        )
                        if q >= 2 and "u" in phases:
                            emit_u_quad(q - 2)
                    if "u" not in phases:
                        continue
                    emit_u_quad(NQ - 2)
                    emit_u_quad(NQ - 1)
                    for dc in range(DC):
                        nc.vector.tensor_copy(
                            U[:, dc, 4 * sb : 4 * sb + 4, :], pu[dc][:]
                        )
                    # Esum slab tree (bf16)
                    T8 = work.tile([P, 8, SB], bf16, tag="T8")
                    T4 = work.tile([P, 4, SB], bf16, tag="T4")
                    T2 = work.tile([P, 2, SB], bf16, tag="T2")
                    nc.vector.tensor_add(T8[:], Eb[:, 0:8, :], Eb[:, 8:16, :])
                    nc.vector.tensor_add(T4[:], T8[:, 0:4, :], T8[:, 4:8, :])
                    nc.vector.tensor_add(T2[:], T4[:, 0:2, :], T4[:, 2:4, :])
                    nc.vector.tensor_add(
                        EsumB[:, 4 * sb : 4 * sb + 4, :], T2[:, 0, :], T2[:, 1, :]
                    )
                    # next rep's transpose groups, spread to fill PE gaps
                    if ems_next and sb >= 1:
                        a, b = {1: (0, 3), 2: (3, 6), 3: (6, 8)}[sb]
                        for _em in ems_next[a:b]:
                            _em()

                if _rep + 1 < reps:
                    tp_cur = tp_next
                    ft_cur = ft_next

                outS = outp.tile([P, NT, D], f32, tag="outS")
                if "p" not in phases:
                    nc.any.memset(outS[:], 0.1)
                    nc.sync.dma_start(
                        out_d.rearrange("(p n) d -> p n d", p=P), outS[:]
                    )
                    continue

                # denominator: one 16-matmul group into a pu-ring bank
                ytmp = ps_u.tile([P, SB], f32, tag="pu")
                for c in range(NT):
                    nc.tensor.matmul(
                        ytmp[:, c : c + 1],
                        EsumB[:, c, :],
                        ones[:],
                        start=(c == 0),
                        stop=(c == NT - 1),
                    )
                nc.vector.reciprocal(rden[:], ytmp[:, 0:NT])

                # phase 3: out rows s=p*16+c: tanh(U_c^T@W1 * rden + dec_c@W2)
                # c-chunks processed in pairs: one Tanh (N=512) per pair.
                def emit_y(c):
                    y = ps_u.tile([P, SB], f32, tag="pu")  # y1 | y2, one group
                    for dc in range(DC):
                        nc.tensor.matmul(
                            y[:, 0:D],
                            U[:, dc, c, :],
                            Wt1[:, dc, :],
                            start=(dc == 0),
                            stop=False,
                        )
                    for dc in range(DC):
                        nc.tensor.matmul(
                            y[:, D : 2 * D],
                            decTi[:, dc, c, :],
                            Wt2[:, dc, :],
                            start=False,
                            stop=(dc == DC - 1),
                        )
                    return y

                for k in range(NT // 2):
                    ya = emit_y(2 * k)
                    yb = emit_y(2 * k + 1)
                    t1 = work.tile([P, 2, D], f32, tag="t1")
                    nc.vector.tensor_scalar_mul(
                        t1[:, 0, :], ya[:, 0:D], rden[:, 2 * k : 2 * k + 1]
                    )
                    nc.vector.tensor_scalar_mul(
                        t1[:, 1, :], yb[:, 0:D], rden[:, 2 * k + 1 : 2 * k + 2]
                    )
                    t2 = work.tile([P, 2, D], f32, tag="t2")
                    nc.vector.tensor_add(t2[:, 0, :], t1[:, 0, :], ya[:, D : 2 * D])
                    nc.vector.tensor_add(t2[:, 1, :], t1[:, 1, :], yb[:, D : 2 * D])
                    nc.scalar.activation(
                        outS[:, 2 * k : 2 * k + 2, :], t2[:], AF.Tanh, bias=zbias[:]
                    )
                nc.sync.dma_start(
                    out_d.rearrange("(p n) d -> p n d", p=P), outS[:]
                )

    nc.compile()
    return nc


def get_nc():
    if "nc" not in _CACHE:
        _CACHE["nc"] = _build()
    return _CACHE["nc"]


def _get_fn():
    if "fn" in _CACHE:
        return _CACHE["fn"]
    import jax
    from jax.sharding import Mesh, NamedSharding, PartitionSpec
    from jax.experimental.shard_map import shard_map
    from concourse.bass2jax import (
        _bass_exec_p,
        install_neuronx_cc_hook,
        partition_id_tensor,
    )

    install_neuronx_cc_hook()
    nc = get_nc()
    out_avals = []
    for alloc in nc.m.functions[0].allocations:
        if (
            isinstance(alloc, mybir.MemoryLocationSet)
            and alloc.kind == "ExternalOutput"
        ):
            out_avals.append(
                jax.core.ShapedArray(
                    tuple(alloc.tensor_shape), mybir.dt.np(alloc.dtype)
                )
            )
    has_pid = nc.partition_id_tensor is not None
    names = ["enc", "dec", "w", "out"] + (["partition_id"] if has_pid else [])
    mesh = Mesh(np.asarray(jax.devices()[:B]), ("core",))
    spec = PartitionSpec("core")

    def _b(e, d, ww, z):
        ops = [e, d, ww, z] + ([partition_id_tensor()] if has_pid else [])
        return _bass_exec_p.bind(
            *ops,
            out_avals=tuple(out_avals),
            in_names=tuple(names),
            out_names=("out",),
            lowering_input_output_aliases=(),
            sim_require_finite=True,
            sim_require_nnan=True,
            nc=nc,
        )[0]

    jitted = jax.jit(
        shard_map(
            _b, mesh=mesh, in_specs=(spec,) * 4, out_specs=spec, check_rep=False
        ),
        donate_argnums=(3,),
        keep_unused=True,
    )
    sh = NamedSharding(mesh, spec)
    _CACHE["fn"] = (jitted, sh)
    return _CACHE["fn"]


def kernel(enc_outputs_top, dec_outputs_top, W_tanh):
    import jax

    enc = np.ascontiguousarray(enc_outputs_top, dtype=np.float32)
    dec = np.ascontiguousarray(dec_outputs_top, dtype=np.float32)
    w = np.ascontiguousarray(W_tanh, dtype=np.float32)
    try:
        fn, sh = _get_fn()
        eg = jax.device_put(enc.reshape(B * S, D), sh)
        dg = jax.device_put(dec.reshape(B * S, D), sh)
        wg = jax.device_put(np.concatenate([w] * B, axis=0), sh)
        zg = jax.device_put(np.zeros((B * S, D), np.float32), sh)
        out = np.asarray(jax.block_until_ready(fn(eg, dg, wg, zg)))
        return out.reshape(B, S, D)
    except Exception:
        nc = get_nc()
        in_maps = [{"enc": enc[b], "dec": dec[b], "w": w} for b in range(B)]
        res = bass_utils.run_bass_kernel_spmd(nc, in_maps, core_ids=list(range(B)))
        return np.stack([r["out"] for r in res.results], axis=0)
